# revision 28
# baseline (speedup 1.0000x reference)
"""Trainium2 Bass kernel for nn_EquivariantScalar (segment_reduce).

Strategy (8 NeuronCores, atom-dim sharding):
  - 200000 atoms split 25000/core, zero-padded to 25088 = 49 tiles x 512,
    processed in 7 groups of 7 tiles.
  - All data fp16 (inputs, weights, intermediates); PSUM accumulates fp32.
  - ACT-table discipline: Sqrt and Silu live in different ACT table sets
    (~1.3us load per switch), so sqrt-ops and silu-ops are batched into
    per-group phases via a pinned dependency chain: 2 loads per group
    instead of 2 per tile.
  - Block-0 squares on ACT (fused (x+eps)^2 via bias), block-1 squares as
    DVE self-multiplies, component-sum adds on the otherwise-idle GPSIMD.
  - sg path folded on host: h1b = (wh1s_1 @ wsg_0) @ g2a + fused biases,
    killing one matmul + one PSUM evacuation per tile.
  - Masked pooling: z from PE (g2b slices vs weff), then fp16 matmul vs
    one-hot batch rows accumulating into one persistent PSUM bank over
    all 49 tiles; host sums the 8 per-core partials.
"""
import os
import sys
import types

import numpy as np

NA = 200000
B = 256
NF = 128
NCORES = 8
SH = NA // NCORES          # 25000 atoms per core
T = 512                    # atoms per tile
NT = 49                    # tiles per core
NAs = NT * T               # 25088 padded atoms per core
G = 7                      # tiles per ACT-phase group
NG = NT // G               # 7 groups
GT = G * T                 # atoms per group
EPS = 1e-8

_prog_cache = {}


def _install_compat_patches():
    """Patches for this container: split multi-wait instructions (this
    walrus caps non-EventSemaphore instructions at ONE sync wait)."""
    import bass_rust
    from concourse import tile
    from concourse.vector_clock import ScopedClock

    if getattr(tile.TileContext, "_wait_split_patched", False):
        return

    def _patched_drain_and_barrier(self, tick_clock, wait_clock):
        nc = self.nc
        drain_inst = nc.sync.drain()
        wait_clock.add_sem_waits(
            drain_inst.ins, ScopedClock({None: tick_clock.global_clock})
        )
        si = drain_inst.ins.sync_info
        if si is not None and len(si.on_wait) > 1:
            waits = list(si.on_wait)
            si.on_wait = waits[:1]
            for w in waits[1:]:
                n = nc.sync.nop(nofuse=True, hint="tail_drain_wait_split")
                n.ins.sync_info = bass_rust.SyncInfo(on_wait=[w], on_update=[])
        nc.all_engine_barrier()
        assert self.sems is not None
        popped = nc._tile_sem_poison_stack.pop()
        assert popped is self._sem_poison
        nc.clear_and_free_semaphores(list(self.sems.allocated().values()))
        nc.all_engine_barrier()

    tile.TileContext._drain_and_barrier = _patched_drain_and_barrier
    tile.TileContext._wait_split_patched = True


def _legalize_waits(nc):
    """Hoist extra sync waits (beyond the per-instruction HW capacity)
    onto fresh single-wait NoOps inserted just before, same engine."""
    import bass_rust
    import concourse.mybir as mybir

    counter = [0]
    for fn in nc.m.functions:
        for bb in fn.blocks:
            out = []
            changed = False
            for inst in bb.instructions:
                si = getattr(inst, "sync_info", None)
                waits = list(si.on_wait) if si is not None else []
                cap = 2 if isinstance(inst, mybir.InstEventSemaphore) else 1
                if len(waits) > cap:
                    si.on_wait = waits[:cap]
                    for w in waits[cap:]:
                        counter[0] += 1
                        n = mybir.InstNoOp(name=f"waitsplit-{counter[0]}")
                        n.engine = inst.engine
                        n.sync_info = bass_rust.SyncInfo(on_wait=[w], on_update=[])
                        out.append(n)
                    changed = True
                out.append(inst)
            if changed:
                bb.instructions = out


def _maybe_install_trace_shim():
    """Optional: NTFF profiling under axon needs antenv.axon_hooks."""
    try:
        import antenv

        if "antenv.axon_hooks" in sys.modules:
            return
        mod = types.ModuleType("antenv.axon_hooks")
        hook = [None]
        mod.set_axon_ntff_profile_hook = lambda h: hook.__setitem__(0, h)
        mod.get_axon_ntff_profile_hook = lambda: hook[0]
        sys.modules["antenv.axon_hooks"] = mod
        antenv.axon_hooks = mod
        from trn_agent_boot.trn_boot import _ntff_profile_via_ctypes

        mod.set_axon_ntff_profile_hook(
            _ntff_profile_via_ctypes("/opt/axon/libaxon_pjrt.so")
        )
    except Exception:
        pass


def _build_program():
    import concourse.bass as bass
    import concourse.mybir as mybir
    from concourse.tile import TileContext
    from concourse.tile_rust import add_dep_helper

    F = mybir.dt.float32
    F16 = mybir.dt.float16
    AF = mybir.ActivationFunctionType

    nc = bass.Bass()

    vT = nc.dram_tensor("vT", [NF, 3, NAs], F16, kind="ExternalInput")
    sT = nc.dram_tensor("sT", [NF, NAs], F16, kind="ExternalInput")
    bT = nc.dram_tensor("bT", [NAs, B], F16, kind="ExternalInput")

    wnames = [
        "wv1_0", "wv2_0", "wh1s_0", "wh1n_0", "wss_0",
        "wv2_1", "wh1sg_1", "wh1n_1",
    ]
    wdram = {n: nc.dram_tensor(n, [NF, NF], F16, kind="ExternalInput") for n in wnames}
    weff_d = nc.dram_tensor("weff", [NF, 1], F16, kind="ExternalInput")
    bias_names = ["b1_0", "bss_0", "b1_1", "beff", "eps"]
    bdram = {n: nc.dram_tensor(n, [NF, 1], F, kind="ExternalInput") for n in bias_names}

    y = nc.dram_tensor("y", [1, B], F, kind="ExternalOutput")

    from contextlib import ExitStack

    with TileContext(nc) as tc:
        with ExitStack() as _stk:
            _p = lambda **kw: _stk.enter_context(tc.tile_pool(**kw))
            wp = _p(name="wp", bufs=1)
            vin_p = _p(name="vin", bufs=2)       # [128,3,GT] f16 = 21KB/p
            sin_p = _p(name="sin", bufs=2)       # [128,GT]  f16 = 7KB/p
            bin_p = _p(name="bin", bufs=G + 1)   # [128,4,B] f16 = 2KB/p
            sqa_p = _p(name="sqa", bufs=3)       # [128,3,T] f16 = 3KB/p
            sqb_p = _p(name="sqb", bufs=3)
            n2a_p = _p(name="n2a", bufs=G + 2)   # [128,T] f16
            n2b_p = _p(name="n2b", bufs=G + 2)
            nrm_p = _p(name="nrm", bufs=G + 1)
            n01_p = _p(name="n01", bufs=4)
            g2a_p = _p(name="g2a", bufs=2 * G + 2)
            g2b_p = _p(name="g2b", bufs=G + 1)
            ss_p = _p(name="ssb", bufs=4)
            vn_p = _p(name="vnw", bufs=3)        # [128,3,T] f16
            zz_p = _p(name="zz", bufs=G + 1)
            yo_p = _p(name="yo", bufs=1)
            psV = _p(name="psV", bufs=4, space="PSUM")
            psH = _p(name="psH", bufs=3, space="PSUM")
            psY = _p(name="psY", bufs=1, space="PSUM")
            w = {}
            for n in wnames:
                w[n] = wp.tile([NF, NF], F16, tag=n, name=n)
                nc.sync.dma_start(out=w[n][:], in_=wdram[n][:])
            weff = wp.tile([NF, 1], F16, tag="weff")
            nc.sync.dma_start(out=weff[:], in_=weff_d[:])
            bias = {}
            for n in bias_names:
                bias[n] = wp.tile([NF, 1], F, tag=n, name=n)
                nc.sync.dma_start(out=bias[n][:], in_=bdram[n][:])

            # Table-set phase fences: Sqrt and Silu live in different ACT
            # table sets, so phase k's ops must all precede phase k+1's on
            # the ACT queue (2 table loads per group instead of per tile).
            # A NOP fence between phases keeps within-phase order free for
            # the scheduler (rigid linear chains deadlock against the
            # PSUM/SBUF slot allocator).
            _fence = [None]
            _phase = [[]]

            def _pin(inst):
                if _fence[0] is not None:
                    add_dep_helper(
                        inst.ins, _fence[0].ins, sync=False,
                        reason="ACT table-set phase order",
                    )
                _phase[0].append(inst)
                return inst

            def _end_phase():
                if not _phase[0]:
                    return
                fence = nc.scalar.nop(nofuse=True, hint="act_phase_fence")
                for i in _phase[0]:
                    add_dep_helper(
                        fence.ins, i.ins, sync=False,
                        reason="ACT table-set phase fence",
                    )
                _fence[0] = fence
                _phase[0] = []

            ypart = psY.tile([1, B], F, tag="ypart", name="ypart")
            pool_mm = [0]  # counts pooling matmuls issued (4 * NT total)

            S = {}  # per-tile state, keyed by global tile index

            def stage_pre(t):
                """DMA-backed block-0 head: v2, squares, c-sum; h1s."""
                g, ti = divmod(t, G)
                st = S[t]
                vt = st["vg"][:, :, ti * T : (ti + 1) * T]
                sq = sqa_p.tile([128, 3, T], F16, tag="sqa", name=f"sqa{t}")
                for c in range(3):
                    pv = psV.tile([128, T], F, tag="v", name=f"pv2a{t}_{c}")
                    nc.tensor.matmul(pv[:], w["wv2_0"][:], vt[:, c, :],
                                     start=True, stop=True)
                    nc.scalar.activation(sq[:, c, :], pv[:], AF.Square,
                                         bias=bias["eps"][:])
                n01 = n01_p.tile([128, T], F16, tag="n01", name=f"n01a{t}")
                nc.gpsimd.tensor_add(n01[:], sq[:, 0, :], sq[:, 1, :])
                n2 = n2a_p.tile([128, T], F16, tag="n2a", name=f"n2a{t}")
                nc.gpsimd.tensor_add(n2[:], n01[:], sq[:, 2, :])
                st["n2a"] = n2

            def stage_block0_tail(t):
                """After silu(g2a): ss, gating, v2b, squares, c-sum."""
                st = S[t]
                g, ti = divmod(t, G)
                vt = st["vg"][:, :, ti * T : (ti + 1) * T]
                pss = psH.tile([128, T], F, tag="h", name=f"pss{t}")
                nc.tensor.matmul(pss[:], w["wss_0"][:], st["g2a"][:],
                                 start=True, stop=True)
                ss = ss_p.tile([128, T], F16, tag="ss", name=f"ss{t}")
                nc.vector.tensor_scalar_add(ss[:], pss[:], bias["bss_0"][:])
                vnew = vn_p.tile([128, 3, T], F16, tag="vn", name=f"vn{t}")
                qb = sqb_p.tile([128, 2, T], F16, tag="qb", name=f"qb{t}")
                sq = sqb_p.tile([128, 3, T], F16, tag="sqb", name=f"sqb{t}")
                for c in range(3):
                    pv1 = psV.tile([128, T], F, tag="v", name=f"pv1{t}_{c}")
                    nc.tensor.matmul(pv1[:], w["wv1_0"][:], vt[:, c, :],
                                     start=True, stop=True)
                    nc.vector.tensor_mul(vnew[:, c, :], pv1[:], ss[:])
                for c in range(3):
                    pv2 = psV.tile([128, T], F, tag="v", name=f"pv2b{t}_{c}")
                    nc.tensor.matmul(pv2[:], w["wv2_1"][:], vnew[:, c, :],
                                     start=True, stop=True)
                    if c == 0:
                        nc.scalar.activation(sq[:, 0, :], pv2[:], AF.Square)
                    else:
                        nc.vector.tensor_copy(qb[:, c - 1, :], pv2[:])
                nc.vector.tensor_mul(sq[:, 1:3, :], qb[:], qb[:])
                n01 = n01_p.tile([128, T], F16, tag="n01", name=f"n01b{t}")
                nc.gpsimd.tensor_add(n01[:], sq[:, 0, :], sq[:, 1, :])
                n2 = n2b_p.tile([128, T], F16, tag="n2b", name=f"n2b{t}")
                nc.vector.tensor_add(n2[:], n01[:], sq[:, 2, :])
                st["n2b"] = n2

            def stage_finish(t):
                """After silu(g2b): z and masked-pool accumulate."""
                st = S[t]
                pz = psH.tile([128, 4], F, tag="h", name=f"pz{t}")
                for j in range(4):
                    nc.tensor.matmul(
                        pz[:, j : j + 1],
                        st["g2b"][:, j * 128 : (j + 1) * 128],
                        weff[:],
                        start=True, stop=True, skip_group_check=True,
                    )
                zt = zz_p.tile([128, 4], F16, tag="zt", name=f"zt{t}")
                nc.vector.tensor_scalar_add(zt[:], pz[:], bias["beff"][:])
                for j in range(4):
                    nc.tensor.matmul(
                        ypart[:],
                        zt[:, j : j + 1],
                        st["bt"][:, j, :],
                        start=(pool_mm[0] == 0),
                        stop=(pool_mm[0] == 4 * NT - 1),
                        skip_group_check=True,
                    )
                    pool_mm[0] += 1
                del S[t]["g2b"]
                del S[t]["bt"]
                del S[t]

            # ---- main loop: one iteration per group, software-pipelined ----
            for g in range(NG + 1):
                tb = list(range(g * G, (g + 1) * G)) if g < NG else []
                tcq = list(range((g - 1) * G, g * G)) if g > 0 else []

                if tb:
                    a0 = g * GT
                    vg = vin_p.tile([128, 3, GT], F16, tag="vg", name=f"vg{g}")
                    nc.sync.dma_start(out=vg[:], in_=vT[:, :, a0 : a0 + GT])
                    sg_in = sin_p.tile([128, GT], F16, tag="sg_in", name=f"sg{g}")
                    nc.sync.dma_start(out=sg_in[:], in_=sT[:, a0 : a0 + GT])
                    for t in tb:
                        S[t] = {"vg": vg, "sg_in": sg_in}
                for t in tcq:
                    bt = bin_p.tile([128, 4, B], F16, tag="bt", name=f"bt{t}")
                    nc.sync.dma_start(
                        out=bt[:],
                        in_=bT[t * T : (t + 1) * T, :].rearrange(
                            "(j p) b -> p j b", p=128
                        ),
                    )
                    S[t]["bt"] = bt

                for t in tb:
                    stage_pre(t)

                # ---- sqrt phase (one table load) ----
                norms0, norms1 = {}, {}
                for t in tb:
                    nm = nrm_p.tile([128, T], F16, tag="nrm0", name=f"nrm0_{t}")
                    _pin(nc.scalar.activation(nm[:], S[t]["n2a"][:], AF.Sqrt))
                    norms0[t] = nm
                for t in tcq:
                    nm = nrm_p.tile([128, T], F16, tag="nrm1", name=f"nrm1_{t}")
                    _pin(nc.scalar.activation(nm[:], S[t]["n2b"][:], AF.Sqrt))
                    norms1[t] = nm
                _end_phase()

                # ---- h1 matmuls + silu phase (one table load) ----
                # B/C interleaved per index so psH slots recycle tilewise
                for k in range(G):
                    if k < len(tb):
                        t = tb[k]
                        g_, ti = divmod(t, G)
                        stt = S[t]["sg_in"][:, ti * T : (ti + 1) * T]
                        ph = psH.tile([128, T], F, tag="h", name=f"ph1a{t}")
                        nc.tensor.matmul(ph[:], w["wh1s_0"][:], stt,
                                         start=True, stop=False,
                                         skip_group_check=True)
                        nc.tensor.matmul(ph[:], w["wh1n_0"][:], norms0[t][:],
                                         start=False, stop=True,
                                         skip_group_check=True)
                        g2a = g2a_p.tile([128, T], F16, tag="g2a", name=f"g2a{t}")
                        _pin(nc.scalar.activation(g2a[:], ph[:], AF.Silu,
                                                  bias=bias["b1_0"][:]))
                        S[t]["g2a"] = g2a
                    if k < len(tcq):
                        t = tcq[k]
                        ph = psH.tile([128, T], F, tag="h", name=f"ph1b{t}")
                        nc.tensor.matmul(ph[:], w["wh1sg_1"][:], S[t]["g2a"][:],
                                         start=True, stop=False,
                                         skip_group_check=True)
                        nc.tensor.matmul(ph[:], w["wh1n_1"][:], norms1[t][:],
                                         start=False, stop=True,
                                         skip_group_check=True)
                        g2b = g2b_p.tile([128, T], F16, tag="g2b", name=f"g2b{t}")
                        _pin(nc.scalar.activation(g2b[:], ph[:], AF.Silu,
                                                  bias=bias["b1_1"][:]))
                        S[t]["g2b"] = g2b
                _end_phase()

                # ---- tails ----
                for t in tcq:
                    stage_finish(t)
                for t in tb:
                    stage_block0_tail(t)

            yout = yo_p.tile([1, B], F, tag="yout", name="yout")
            nc.vector.tensor_copy(yout[:], ypart[:])
            nc.sync.dma_start(out=y[:], in_=yout[:])

    _legalize_waits(nc)
    return nc


def _prep_weights(inputs):
    f32 = lambda a: np.asarray(a, np.float32)
    u0_w, v0_w = f32(inputs["u0_w"]), f32(inputs["v0_w"])
    a0_w1, a0_b1 = f32(inputs["a0_w1"]), f32(inputs["a0_b1"])
    a0_w2, a0_b2 = f32(inputs["a0_w2"]), f32(inputs["a0_b2"])
    v1_w = f32(inputs["v1_w"])
    a1_w1, a1_b1 = f32(inputs["a1_w1"]), f32(inputs["a1_b1"])
    a1_w2, a1_b2 = f32(inputs["a1_w2"]), f32(inputs["a1_b2"])
    out_w, out_b = f32(inputs["out_w"]), f32(inputs["out_b"])

    f16c = lambda a: np.ascontiguousarray(a, np.float16)
    f32c = lambda a: np.ascontiguousarray(a.reshape(NF, 1), np.float32)

    # composed block-1 s-path: h1b_sg = (W1s @ Wsg) @ g2a ; bias folded
    W1s = a1_w1[:, :NF]                  # (g_out, f_in) acting on block-1 s
    Wsg = a0_w2[:NF, :]                  # (f_out, j_in) block-0 sg head
    bsg = a0_b2[:NF]
    Wc = W1s @ Wsg                       # (g_out, j_in)
    b1_1_eff = a1_b1 + W1s @ bsg

    return {
        "wv1_0": f16c(u0_w.T),
        "wv2_0": f16c(v0_w.T),
        "wh1s_0": f16c(a0_w1.T[:NF]),
        "wh1n_0": f16c(a0_w1.T[NF:]),
        "wss_0": f16c(a0_w2[NF:].T),
        "wv2_1": f16c(v1_w.T),
        "wh1sg_1": f16c(Wc.T),
        "wh1n_1": f16c(a1_w1.T[NF:]),
        "weff": f16c((out_w[0] @ a1_w2[:NF]).reshape(NF, 1)),
        "b1_0": f32c(np.broadcast_to(a0_b1, (NF,)).copy()),
        "bss_0": f32c(a0_b2[NF:].copy()),
        "b1_1": f32c(b1_1_eff),
        "beff": np.full((NF, 1), float(out_w[0] @ a1_b2[:NF] + out_b[0]), np.float32),
        "eps": np.full((NF, 1), EPS, np.float32),
    }


def kernel(**inputs):
    _install_compat_patches()
    if os.environ.get("BASS_TRACE"):
        _maybe_install_trace_shim()
    from concourse.bass_utils import run_bass_kernel_spmd

    s = np.asarray(inputs["s"], np.float32)
    v = np.asarray(inputs["v"], np.float32)
    batch = np.asarray(inputs["batch"], np.float32)
    weights = _prep_weights(inputs)

    v0 = v[0]            # (NA, 3, NF)
    s0 = s[0]            # (NA, NF)
    bm = batch[:, :, 0]  # (B, NA)

    in_maps = []
    for c in range(NCORES):
        sl = slice(c * SH, (c + 1) * SH)
        vt = np.zeros((NF, 3, NAs), np.float16)
        vt[:, :, :SH] = v0[sl].transpose(2, 1, 0)
        st = np.zeros((NF, NAs), np.float16)
        st[:, :SH] = s0[sl].T
        bt = np.zeros((NAs, B), np.float16)
        bt[:SH] = bm[:, sl].T
        in_maps.append({"vT": vt, "sT": st, "bT": bt, **weights})

    key = "prog"
    if key not in _prog_cache:
        _prog_cache[key] = _build_program()
    nc = _prog_cache[key]

    res = run_bass_kernel_spmd(nc, in_maps, list(range(NCORES)))
    if res.exec_time_ns is not None:
        print(f"HW exec time: {res.exec_time_ns} ns")
    kernel._last_result = res

    ysum = np.zeros((B,), np.float64)
    for c in range(NCORES):
        ysum += res.results[c]["y"].reshape(B).astype(np.float64)
    return ysum.astype(np.float32).reshape(B, 1)


# revision 30
# speedup vs baseline: 1.0782x; 1.0782x over previous
"""Trainium2 Bass kernel for nn_EquivariantScalar (segment_reduce).

Strategy (8 NeuronCores, atom-dim sharding):
  - 200000 atoms split 25000/core, zero-padded to 25088 = 49 tiles x 512,
    processed in 7 groups of 7 tiles.
  - All data fp16 (inputs, weights, intermediates); PSUM accumulates fp32.
  - ACT-table discipline: Sqrt and Silu live in different ACT table sets
    (~1.3us load per switch), so sqrt-ops and silu-ops are batched into
    per-group phases via a pinned dependency chain: 2 loads per group
    instead of 2 per tile.
  - Block-0 squares on ACT (fused (x+eps)^2 via bias), block-1 squares as
    DVE self-multiplies, component-sum adds on the otherwise-idle GPSIMD.
  - sg path folded on host: h1b = (wh1s_1 @ wsg_0) @ g2a + fused biases,
    killing one matmul + one PSUM evacuation per tile.
  - Masked pooling: z from PE (g2b slices vs weff), then fp16 matmul vs
    one-hot batch rows accumulating into one persistent PSUM bank over
    all 49 tiles; host sums the 8 per-core partials.
"""
import os
import sys
import types

import numpy as np

NA = 200000
B = 256
NF = 128
NCORES = 8
SH = NA // NCORES          # 25000 atoms per core
T = 512                    # atoms per tile
NT = 49                    # tiles per core
NAs = NT * T               # 25088 padded atoms per core
G = 7                      # tiles per ACT-phase group
NG = NT // G               # 7 groups
GT = G * T                 # atoms per group
EPS = 1e-8

_prog_cache = {}


def _install_compat_patches():
    """Patches for this container: split multi-wait instructions (this
    walrus caps non-EventSemaphore instructions at ONE sync wait)."""
    import bass_rust
    from concourse import tile
    from concourse.vector_clock import ScopedClock

    if getattr(tile.TileContext, "_wait_split_patched", False):
        return

    def _patched_drain_and_barrier(self, tick_clock, wait_clock):
        nc = self.nc
        drain_inst = nc.sync.drain()
        wait_clock.add_sem_waits(
            drain_inst.ins, ScopedClock({None: tick_clock.global_clock})
        )
        si = drain_inst.ins.sync_info
        if si is not None and len(si.on_wait) > 1:
            waits = list(si.on_wait)
            si.on_wait = waits[:1]
            for w in waits[1:]:
                n = nc.sync.nop(nofuse=True, hint="tail_drain_wait_split")
                n.ins.sync_info = bass_rust.SyncInfo(on_wait=[w], on_update=[])
        nc.all_engine_barrier()
        assert self.sems is not None
        popped = nc._tile_sem_poison_stack.pop()
        assert popped is self._sem_poison
        nc.clear_and_free_semaphores(list(self.sems.allocated().values()))
        nc.all_engine_barrier()

    tile.TileContext._drain_and_barrier = _patched_drain_and_barrier
    tile.TileContext._wait_split_patched = True


def _legalize_waits(nc):
    """Hoist extra sync waits (beyond the per-instruction HW capacity)
    onto fresh single-wait NoOps inserted just before, same engine."""
    import bass_rust
    import concourse.mybir as mybir

    counter = [0]
    for fn in nc.m.functions:
        for bb in fn.blocks:
            out = []
            changed = False
            for inst in bb.instructions:
                si = getattr(inst, "sync_info", None)
                waits = list(si.on_wait) if si is not None else []
                cap = 2 if isinstance(inst, mybir.InstEventSemaphore) else 1
                if len(waits) > cap:
                    si.on_wait = waits[:cap]
                    for w in waits[cap:]:
                        counter[0] += 1
                        n = mybir.InstNoOp(name=f"waitsplit-{counter[0]}")
                        n.engine = inst.engine
                        n.sync_info = bass_rust.SyncInfo(on_wait=[w], on_update=[])
                        out.append(n)
                    changed = True
                out.append(inst)
            if changed:
                bb.instructions = out


def _maybe_install_trace_shim():
    """Optional: NTFF profiling under axon needs antenv.axon_hooks."""
    try:
        import antenv

        if "antenv.axon_hooks" in sys.modules:
            return
        mod = types.ModuleType("antenv.axon_hooks")
        hook = [None]
        mod.set_axon_ntff_profile_hook = lambda h: hook.__setitem__(0, h)
        mod.get_axon_ntff_profile_hook = lambda: hook[0]
        sys.modules["antenv.axon_hooks"] = mod
        antenv.axon_hooks = mod
        from trn_agent_boot.trn_boot import _ntff_profile_via_ctypes

        mod.set_axon_ntff_profile_hook(
            _ntff_profile_via_ctypes("/opt/axon/libaxon_pjrt.so")
        )
    except Exception:
        pass


def _build_program():
    import concourse.bass as bass
    import concourse.mybir as mybir
    from concourse.tile import TileContext
    from concourse.tile_rust import add_dep_helper

    F = mybir.dt.float32
    F16 = mybir.dt.float16
    AF = mybir.ActivationFunctionType

    nc = bass.Bass()

    vT = nc.dram_tensor("vT", [NF, 3, NAs], F16, kind="ExternalInput")
    sT = nc.dram_tensor("sT", [NF, NAs], F16, kind="ExternalInput")
    bT = nc.dram_tensor("bT", [NAs, B], F16, kind="ExternalInput")

    wnames = [
        "wv1_0", "wv2_0", "wh1s_0", "wh1n_0", "wss_0",
        "wv2_1", "wh1sg_1", "wh1n_1",
    ]
    wdram = {n: nc.dram_tensor(n, [NF, NF], F16, kind="ExternalInput") for n in wnames}
    weff_d = nc.dram_tensor("weff", [NF, 1], F16, kind="ExternalInput")
    bias_names = ["b1_0", "bss_0", "b1_1", "beff", "eps"]
    bdram = {n: nc.dram_tensor(n, [NF, 1], F, kind="ExternalInput") for n in bias_names}

    y = nc.dram_tensor("y", [1, B], F, kind="ExternalOutput")

    from contextlib import ExitStack

    with TileContext(nc) as tc:
        with ExitStack() as _stk:
            _p = lambda **kw: _stk.enter_context(tc.tile_pool(**kw))
            wp = _p(name="wp", bufs=1)
            vin_p = _p(name="vin", bufs=2)       # [128,3,GT] f16 = 21KB/p
            sin_p = _p(name="sin", bufs=2)       # [128,GT]  f16 = 7KB/p
            bin_p = _p(name="bin", bufs=G + 1)   # [128,4,B] f16 = 2KB/p
            sqa_p = _p(name="sqa", bufs=3)       # [128,3,T] f16 = 3KB/p
            sqb_p = _p(name="sqb", bufs=3)
            n2a_p = _p(name="n2a", bufs=G + 2)   # [128,T] f16
            n2b_p = _p(name="n2b", bufs=G + 2)
            nrm_p = _p(name="nrm", bufs=G + 1)
            n01_p = _p(name="n01", bufs=4)
            g2a_p = _p(name="g2a", bufs=2 * G + 2)
            g2b_p = _p(name="g2b", bufs=G + 1)
            ss_p = _p(name="ssb", bufs=4)
            vn_p = _p(name="vnw", bufs=3)        # [128,3,T] f16
            zz_p = _p(name="zz", bufs=G + 1)
            yo_p = _p(name="yo", bufs=1)
            psV = _p(name="psV", bufs=2, space="PSUM")   # block-0 v2a heads
            psB = _p(name="psB", bufs=2, space="PSUM")   # block-0 tail v1/v2b
            psH = _p(name="psH", bufs=3, space="PSUM")
            psY = _p(name="psY", bufs=1, space="PSUM")
            w = {}
            for n in wnames:
                w[n] = wp.tile([NF, NF], F16, tag=n, name=n)
                nc.sync.dma_start(out=w[n][:], in_=wdram[n][:])
            weff = wp.tile([NF, 1], F16, tag="weff")
            nc.sync.dma_start(out=weff[:], in_=weff_d[:])
            bias = {}
            for n in bias_names:
                bias[n] = wp.tile([NF, 1], F, tag=n, name=n)
                nc.sync.dma_start(out=bias[n][:], in_=bdram[n][:])

            # Table-set phase fences: Sqrt and Silu live in different ACT
            # table sets, so phase k's ops must all precede phase k+1's on
            # the ACT queue (2 table loads per group instead of per tile).
            # A NOP fence between phases keeps within-phase order free for
            # the scheduler (rigid linear chains deadlock against the
            # PSUM/SBUF slot allocator).
            _fence = [None]
            _phase = [[]]

            def _pin(inst):
                if _fence[0] is not None:
                    add_dep_helper(
                        inst.ins, _fence[0].ins, sync=False,
                        reason="ACT table-set phase order",
                    )
                _phase[0].append(inst)
                return inst

            def _end_phase():
                if not _phase[0]:
                    return
                fence = nc.scalar.nop(nofuse=True, hint="act_phase_fence")
                for i in _phase[0]:
                    add_dep_helper(
                        fence.ins, i.ins, sync=False,
                        reason="ACT table-set phase fence",
                    )
                _fence[0] = fence
                _phase[0] = []

            ypart = psY.tile([1, B], F, tag="ypart", name="ypart")
            pool_mm = [0]  # counts pooling matmuls issued (4 * NT total)

            S = {}  # per-tile state, keyed by global tile index

            def stage_pre(t):
                """DMA-backed block-0 head: v2, squares, c-sum; h1s."""
                g, ti = divmod(t, G)
                st = S[t]
                vt = st["vg"][:, :, ti * T : (ti + 1) * T]
                sq = sqa_p.tile([128, 3, T], F16, tag="sqa", name=f"sqa{t}")
                for c in range(3):
                    pv = psV.tile([128, T], F, tag="v", name=f"pv2a{t}_{c}")
                    nc.tensor.matmul(pv[:], w["wv2_0"][:], vt[:, c, :],
                                     start=True, stop=True)
                    nc.scalar.activation(sq[:, c, :], pv[:], AF.Square,
                                         bias=bias["eps"][:])
                n01 = n01_p.tile([128, T], F16, tag="n01", name=f"n01a{t}")
                nc.gpsimd.tensor_add(n01[:], sq[:, 0, :], sq[:, 1, :])
                n2 = n2a_p.tile([128, T], F16, tag="n2a", name=f"n2a{t}")
                nc.gpsimd.tensor_add(n2[:], n01[:], sq[:, 2, :])
                st["n2a"] = n2

            def stage_block0_tail(t):
                """After silu(g2a): ss, gating, v2b, squares, c-sum."""
                st = S[t]
                g, ti = divmod(t, G)
                vt = st["vg"][:, :, ti * T : (ti + 1) * T]
                pss = psH.tile([128, T], F, tag="h", name=f"pss{t}")
                nc.tensor.matmul(pss[:], w["wss_0"][:], st["g2a"][:],
                                 start=True, stop=True)
                ss = ss_p.tile([128, T], F16, tag="ss", name=f"ss{t}")
                nc.vector.tensor_scalar_add(ss[:], pss[:], bias["bss_0"][:])
                vnew = vn_p.tile([128, 3, T], F16, tag="vn", name=f"vn{t}")
                sq = sqb_p.tile([128, 3, T], F16, tag="sqb", name=f"sqb{t}")
                for c in range(3):
                    pv1 = psB.tile([128, T], F, tag="vb", name=f"pv1{t}_{c}")
                    nc.tensor.matmul(pv1[:], w["wv1_0"][:], vt[:, c, :],
                                     start=True, stop=True)
                    nc.vector.tensor_mul(vnew[:, c, :], pv1[:], ss[:])
                for c in range(3):
                    pv2 = psB.tile([128, T], F, tag="vb", name=f"pv2b{t}_{c}")
                    nc.tensor.matmul(pv2[:], w["wv2_1"][:], vnew[:, c, :],
                                     start=True, stop=True)
                    nc.scalar.activation(sq[:, c, :], pv2[:], AF.Square)
                n01 = n01_p.tile([128, T], F16, tag="n01", name=f"n01b{t}")
                nc.gpsimd.tensor_add(n01[:], sq[:, 0, :], sq[:, 1, :])
                n2 = n2b_p.tile([128, T], F16, tag="n2b", name=f"n2b{t}")
                nc.gpsimd.tensor_add(n2[:], n01[:], sq[:, 2, :])
                st["n2b"] = n2

            def stage_finish(t):
                """After silu(g2b): z and masked-pool accumulate."""
                st = S[t]
                pz = psH.tile([128, 4], F, tag="h", name=f"pz{t}")
                for j in range(4):
                    nc.tensor.matmul(
                        pz[:, j : j + 1],
                        st["g2b"][:, j * 128 : (j + 1) * 128],
                        weff[:],
                        start=True, stop=True, skip_group_check=True,
                    )
                zt = zz_p.tile([128, 4], F16, tag="zt", name=f"zt{t}")
                nc.vector.tensor_scalar_add(zt[:], pz[:], bias["beff"][:])
                for j in range(4):
                    nc.tensor.matmul(
                        ypart[:],
                        zt[:, j : j + 1],
                        st["bt"][:, j, :],
                        start=(pool_mm[0] == 0),
                        stop=(pool_mm[0] == 4 * NT - 1),
                        skip_group_check=True,
                    )
                    pool_mm[0] += 1
                del S[t]["g2b"]
                del S[t]["bt"]
                del S[t]

            # ---- main loop: one iteration per group, software-pipelined ----
            for g in range(NG + 1):
                tb = list(range(g * G, (g + 1) * G)) if g < NG else []
                tcq = list(range((g - 1) * G, g * G)) if g > 0 else []

                if tb:
                    a0 = g * GT
                    vg = vin_p.tile([128, 3, GT], F16, tag="vg", name=f"vg{g}")
                    nc.sync.dma_start(out=vg[:], in_=vT[:, :, a0 : a0 + GT])
                    sg_in = sin_p.tile([128, GT], F16, tag="sg_in", name=f"sg{g}")
                    nc.sync.dma_start(out=sg_in[:], in_=sT[:, a0 : a0 + GT])
                    for t in tb:
                        S[t] = {"vg": vg, "sg_in": sg_in}
                for t in tcq:
                    bt = bin_p.tile([128, 4, B], F16, tag="bt", name=f"bt{t}")
                    nc.sync.dma_start(
                        out=bt[:],
                        in_=bT[t * T : (t + 1) * T, :].rearrange(
                            "(j p) b -> p j b", p=128
                        ),
                    )
                    S[t]["bt"] = bt

                for t in tb:
                    stage_pre(t)

                # ---- sqrt phase (one table load) ----
                norms0, norms1 = {}, {}
                for t in tb:
                    nm = nrm_p.tile([128, T], F16, tag="nrm0", name=f"nrm0_{t}")
                    _pin(nc.scalar.activation(nm[:], S[t]["n2a"][:], AF.Sqrt))
                    norms0[t] = nm
                for t in tcq:
                    nm = nrm_p.tile([128, T], F16, tag="nrm1", name=f"nrm1_{t}")
                    _pin(nc.scalar.activation(nm[:], S[t]["n2b"][:], AF.Sqrt))
                    norms1[t] = nm
                _end_phase()

                # ---- h1 matmuls + silu phase (one table load) ----
                # B/C interleaved per index so psH slots recycle tilewise
                for k in range(G):
                    if k < len(tb):
                        t = tb[k]
                        g_, ti = divmod(t, G)
                        stt = S[t]["sg_in"][:, ti * T : (ti + 1) * T]
                        ph = psH.tile([128, T], F, tag="h", name=f"ph1a{t}")
                        nc.tensor.matmul(ph[:], w["wh1s_0"][:], stt,
                                         start=True, stop=False,
                                         skip_group_check=True)
                        nc.tensor.matmul(ph[:], w["wh1n_0"][:], norms0[t][:],
                                         start=False, stop=True,
                                         skip_group_check=True)
                        g2a = g2a_p.tile([128, T], F16, tag="g2a", name=f"g2a{t}")
                        _pin(nc.scalar.activation(g2a[:], ph[:], AF.Silu,
                                                  bias=bias["b1_0"][:]))
                        S[t]["g2a"] = g2a
                    if k < len(tcq):
                        t = tcq[k]
                        ph = psH.tile([128, T], F, tag="h", name=f"ph1b{t}")
                        nc.tensor.matmul(ph[:], w["wh1sg_1"][:], S[t]["g2a"][:],
                                         start=True, stop=False,
                                         skip_group_check=True)
                        nc.tensor.matmul(ph[:], w["wh1n_1"][:], norms1[t][:],
                                         start=False, stop=True,
                                         skip_group_check=True)
                        g2b = g2b_p.tile([128, T], F16, tag="g2b", name=f"g2b{t}")
                        _pin(nc.scalar.activation(g2b[:], ph[:], AF.Silu,
                                                  bias=bias["b1_1"][:]))
                        S[t]["g2b"] = g2b
                _end_phase()

                # ---- tails ----
                for t in tcq:
                    stage_finish(t)
                for t in tb:
                    stage_block0_tail(t)

            yout = yo_p.tile([1, B], F, tag="yout", name="yout")
            nc.vector.tensor_copy(yout[:], ypart[:])
            nc.sync.dma_start(out=y[:], in_=yout[:])

    _legalize_waits(nc)
    return nc


def _prep_weights(inputs):
    f32 = lambda a: np.asarray(a, np.float32)
    u0_w, v0_w = f32(inputs["u0_w"]), f32(inputs["v0_w"])
    a0_w1, a0_b1 = f32(inputs["a0_w1"]), f32(inputs["a0_b1"])
    a0_w2, a0_b2 = f32(inputs["a0_w2"]), f32(inputs["a0_b2"])
    v1_w = f32(inputs["v1_w"])
    a1_w1, a1_b1 = f32(inputs["a1_w1"]), f32(inputs["a1_b1"])
    a1_w2, a1_b2 = f32(inputs["a1_w2"]), f32(inputs["a1_b2"])
    out_w, out_b = f32(inputs["out_w"]), f32(inputs["out_b"])

    f16c = lambda a: np.ascontiguousarray(a, np.float16)
    f32c = lambda a: np.ascontiguousarray(a.reshape(NF, 1), np.float32)

    # composed block-1 s-path: h1b_sg = (W1s @ Wsg) @ g2a ; bias folded
    W1s = a1_w1[:, :NF]                  # (g_out, f_in) acting on block-1 s
    Wsg = a0_w2[:NF, :]                  # (f_out, j_in) block-0 sg head
    bsg = a0_b2[:NF]
    Wc = W1s @ Wsg                       # (g_out, j_in)
    b1_1_eff = a1_b1 + W1s @ bsg

    return {
        "wv1_0": f16c(u0_w.T),
        "wv2_0": f16c(v0_w.T),
        "wh1s_0": f16c(a0_w1.T[:NF]),
        "wh1n_0": f16c(a0_w1.T[NF:]),
        "wss_0": f16c(a0_w2[NF:].T),
        "wv2_1": f16c(v1_w.T),
        "wh1sg_1": f16c(Wc.T),
        "wh1n_1": f16c(a1_w1.T[NF:]),
        "weff": f16c((out_w[0] @ a1_w2[:NF]).reshape(NF, 1)),
        "b1_0": f32c(np.broadcast_to(a0_b1, (NF,)).copy()),
        "bss_0": f32c(a0_b2[NF:].copy()),
        "b1_1": f32c(b1_1_eff),
        "beff": np.full((NF, 1), float(out_w[0] @ a1_b2[:NF] + out_b[0]), np.float32),
        "eps": np.full((NF, 1), EPS, np.float32),
    }


def kernel(**inputs):
    _install_compat_patches()
    if os.environ.get("BASS_TRACE"):
        _maybe_install_trace_shim()
    from concourse.bass_utils import run_bass_kernel_spmd

    s = np.asarray(inputs["s"], np.float32)
    v = np.asarray(inputs["v"], np.float32)
    batch = np.asarray(inputs["batch"], np.float32)
    weights = _prep_weights(inputs)

    v0 = v[0]            # (NA, 3, NF)
    s0 = s[0]            # (NA, NF)
    bm = batch[:, :, 0]  # (B, NA)

    in_maps = []
    for c in range(NCORES):
        sl = slice(c * SH, (c + 1) * SH)
        vt = np.zeros((NF, 3, NAs), np.float16)
        vt[:, :, :SH] = v0[sl].transpose(2, 1, 0)
        st = np.zeros((NF, NAs), np.float16)
        st[:, :SH] = s0[sl].T
        bt = np.zeros((NAs, B), np.float16)
        bt[:SH] = bm[:, sl].T
        in_maps.append({"vT": vt, "sT": st, "bT": bt, **weights})

    key = "prog"
    if key not in _prog_cache:
        _prog_cache[key] = _build_program()
    nc = _prog_cache[key]

    res = run_bass_kernel_spmd(nc, in_maps, list(range(NCORES)))
    if res.exec_time_ns is not None:
        print(f"HW exec time: {res.exec_time_ns} ns")
    kernel._last_result = res

    ysum = np.zeros((B,), np.float64)
    for c in range(NCORES):
        ysum += res.results[c]["y"].reshape(B).astype(np.float64)
    return ysum.astype(np.float32).reshape(B, 1)


# revision 34
# speedup vs baseline: 1.4041x; 1.3023x over previous
"""Trainium2 Bass kernel for nn_EquivariantScalar (segment_reduce).

Strategy (8 NeuronCores, atom-dim sharding):
  - 200000 atoms split 25000/core, zero-padded to 25088 = 49 tiles x 512,
    processed in 7 groups of 7 tiles.
  - All data fp16 (inputs, weights, intermediates); PSUM accumulates fp32.
  - ACT-table discipline: Sqrt and Silu live in different ACT table sets
    (~1.3us load per switch), so sqrt-ops and silu-ops are batched into
    per-group phases via a pinned dependency chain: 2 loads per group
    instead of 2 per tile.
  - Block-0 squares on ACT (fused (x+eps)^2 via bias), block-1 squares as
    DVE self-multiplies, component-sum adds on the otherwise-idle GPSIMD.
  - sg path folded on host: h1b = (wh1s_1 @ wsg_0) @ g2a + fused biases,
    killing one matmul + one PSUM evacuation per tile.
  - Masked pooling: z from PE (g2b slices vs weff), then fp16 matmul vs
    one-hot batch rows accumulating into one persistent PSUM bank over
    all 49 tiles; host sums the 8 per-core partials.
"""
import os
import sys
import types

import numpy as np

NA = 200000
B = 256
NF = 128
NCORES = 8
SH = NA // NCORES          # 25000 atoms per core
T = 512                    # atoms per tile
NT = 49                    # tiles per core
NAs = NT * T               # 25088 padded atoms per core
G = 7                      # tiles per ACT-phase group
NG = NT // G               # 7 groups
GT = G * T                 # atoms per group
EPS = 1e-8

_prog_cache = {}


def _install_compat_patches():
    """Patches for this container: split multi-wait instructions (this
    walrus caps non-EventSemaphore instructions at ONE sync wait)."""
    import bass_rust
    from concourse import tile
    from concourse.vector_clock import ScopedClock

    if getattr(tile.TileContext, "_wait_split_patched", False):
        return

    def _patched_drain_and_barrier(self, tick_clock, wait_clock):
        nc = self.nc
        drain_inst = nc.sync.drain()
        wait_clock.add_sem_waits(
            drain_inst.ins, ScopedClock({None: tick_clock.global_clock})
        )
        si = drain_inst.ins.sync_info
        if si is not None and len(si.on_wait) > 1:
            waits = list(si.on_wait)
            si.on_wait = waits[:1]
            for w in waits[1:]:
                n = nc.sync.nop(nofuse=True, hint="tail_drain_wait_split")
                n.ins.sync_info = bass_rust.SyncInfo(on_wait=[w], on_update=[])
        nc.all_engine_barrier()
        assert self.sems is not None
        popped = nc._tile_sem_poison_stack.pop()
        assert popped is self._sem_poison
        nc.clear_and_free_semaphores(list(self.sems.allocated().values()))
        nc.all_engine_barrier()

    tile.TileContext._drain_and_barrier = _patched_drain_and_barrier
    tile.TileContext._wait_split_patched = True


def _legalize_waits(nc):
    """Hoist extra sync waits (beyond the per-instruction HW capacity)
    onto fresh single-wait NoOps inserted just before, same engine."""
    import bass_rust
    import concourse.mybir as mybir

    counter = [0]
    for fn in nc.m.functions:
        for bb in fn.blocks:
            out = []
            changed = False
            for inst in bb.instructions:
                si = getattr(inst, "sync_info", None)
                waits = list(si.on_wait) if si is not None else []
                cap = 2 if isinstance(inst, mybir.InstEventSemaphore) else 1
                if len(waits) > cap:
                    si.on_wait = waits[:cap]
                    for w in waits[cap:]:
                        counter[0] += 1
                        n = mybir.InstNoOp(name=f"waitsplit-{counter[0]}")
                        n.engine = inst.engine
                        n.sync_info = bass_rust.SyncInfo(on_wait=[w], on_update=[])
                        out.append(n)
                    changed = True
                out.append(inst)
            if changed:
                bb.instructions = out


def _maybe_install_trace_shim():
    """Optional: NTFF profiling under axon needs antenv.axon_hooks."""
    try:
        import antenv

        if "antenv.axon_hooks" in sys.modules:
            return
        mod = types.ModuleType("antenv.axon_hooks")
        hook = [None]
        mod.set_axon_ntff_profile_hook = lambda h: hook.__setitem__(0, h)
        mod.get_axon_ntff_profile_hook = lambda: hook[0]
        sys.modules["antenv.axon_hooks"] = mod
        antenv.axon_hooks = mod
        from trn_agent_boot.trn_boot import _ntff_profile_via_ctypes

        mod.set_axon_ntff_profile_hook(
            _ntff_profile_via_ctypes("/opt/axon/libaxon_pjrt.so")
        )
    except Exception:
        pass


def _build_program():
    import concourse.bass as bass
    import concourse.mybir as mybir
    from concourse.tile import TileContext
    from concourse.tile_rust import add_dep_helper

    F = mybir.dt.float32
    F16 = mybir.dt.float16
    AF = mybir.ActivationFunctionType

    nc = bass.Bass()

    vT = nc.dram_tensor("vT", [NF, 3, NAs], F16, kind="ExternalInput")
    sT = nc.dram_tensor("sT", [NF, NAs], F16, kind="ExternalInput")
    bT = nc.dram_tensor("bT", [NAs, B], F16, kind="ExternalInput")

    wnames = [
        "wv1_0", "wv2_0", "wh1s_0", "wh1n_0", "wss_0",
        "wv2_1", "wh1sg_1", "wh1n_1",
    ]
    wdram = {n: nc.dram_tensor(n, [NF, NF], F16, kind="ExternalInput") for n in wnames}
    weff_d = nc.dram_tensor("weff", [NF, 1], F16, kind="ExternalInput")
    bias_names = ["b1_0", "bss_0", "b1_1", "beff", "eps"]
    bdram = {n: nc.dram_tensor(n, [NF, 1], F, kind="ExternalInput") for n in bias_names}

    y = nc.dram_tensor("y", [1, B], F, kind="ExternalOutput")

    from contextlib import ExitStack

    with TileContext(nc) as tc:
        with ExitStack() as _stk:
            _p = lambda **kw: _stk.enter_context(tc.tile_pool(**kw))
            wp = _p(name="wp", bufs=1)
            vin_p = _p(name="vin", bufs=3)       # [128,3,GT] f16 = 21KB/p
            sin_p = _p(name="sin", bufs=2)       # [128,GT]  f16 = 7KB/p
            bin_p = _p(name="bin", bufs=G + 1)   # [128,4,B] f16 = 2KB/p
            sqa_p = _p(name="sqa", bufs=4)       # [128,3,T] f16 = 3KB/p
            sqb_p = _p(name="sqb", bufs=3)
            n2a_p = _p(name="n2a", bufs=2 * G + 2)   # [128,T] f16
            n2b_p = _p(name="n2b", bufs=G + 2)
            nrm_p = _p(name="nrm", bufs=G + 1)
            n01_p = _p(name="n01", bufs=4)
            g2a_p = _p(name="g2a", bufs=2 * G + 2)
            g2b_p = _p(name="g2b", bufs=G + 1)
            ss_p = _p(name="ssb", bufs=4)
            vn_p = _p(name="vnw", bufs=3)        # [128,3,T] f16
            zz_p = _p(name="zz", bufs=G + 1)
            yo_p = _p(name="yo", bufs=1)
            psV = _p(name="psV", bufs=4, space="PSUM")
            psH = _p(name="psH", bufs=3, space="PSUM")
            psY = _p(name="psY", bufs=1, space="PSUM")
            w = {}
            for n in wnames:
                w[n] = wp.tile([NF, NF], F16, tag=n, name=n)
                nc.sync.dma_start(out=w[n][:], in_=wdram[n][:])
            weff = wp.tile([NF, 1], F16, tag="weff")
            nc.sync.dma_start(out=weff[:], in_=weff_d[:])
            bias = {}
            for n in bias_names:
                bias[n] = wp.tile([NF, 1], F, tag=n, name=n)
                nc.sync.dma_start(out=bias[n][:], in_=bdram[n][:])

            # Table-set phase fences: Sqrt and Silu live in different ACT
            # table sets, so phase k's ops must all precede phase k+1's on
            # the ACT queue (2 table loads per group instead of per tile).
            # A NOP fence between phases keeps within-phase order free for
            # the scheduler (rigid linear chains deadlock against the
            # PSUM/SBUF slot allocator).
            _fence = [None]
            _phase = [[]]

            def _pin(inst):
                if _fence[0] is not None:
                    add_dep_helper(
                        inst.ins, _fence[0].ins, sync=False,
                        reason="ACT table-set phase order",
                    )
                _phase[0].append(inst)
                return inst

            def _end_phase():
                if not _phase[0]:
                    return
                fence = nc.scalar.nop(nofuse=True, hint="act_phase_fence")
                for i in _phase[0]:
                    add_dep_helper(
                        fence.ins, i.ins, sync=False,
                        reason="ACT table-set phase fence",
                    )
                _fence[0] = fence
                _phase[0] = []

            ypart = psY.tile([1, B], F, tag="ypart", name="ypart")
            pool_mm = [0]  # counts pooling matmuls issued (4 * NT total)

            S = {}  # per-tile state, keyed by global tile index

            def stage_pre(t):
                """DMA-backed block-0 head: v2, squares, c-sum; h1s."""
                g, ti = divmod(t, G)
                st = S[t]
                vt = st["vg"][:, :, ti * T : (ti + 1) * T]
                sq = sqa_p.tile([128, 3, T], F16, tag="sqa", name=f"sqa{t}")
                for c in range(3):
                    pv = psV.tile([128, T], F, tag="v", name=f"pv2a{t}_{c}")
                    nc.tensor.matmul(pv[:], w["wv2_0"][:], vt[:, c, :],
                                     start=True, stop=True)
                    nc.scalar.activation(sq[:, c, :], pv[:], AF.Square,
                                         bias=bias["eps"][:])
                n01 = n01_p.tile([128, T], F16, tag="n01", name=f"n01a{t}")
                nc.gpsimd.tensor_add(n01[:], sq[:, 0, :], sq[:, 1, :])
                n2 = n2a_p.tile([128, T], F16, tag="n2a", name=f"n2a{t}")
                nc.vector.tensor_add(n2[:], n01[:], sq[:, 2, :])
                st["n2a"] = n2

            def stage_block0_tail(t):
                """After silu(g2a): ss, gating, v2b, squares, c-sum."""
                st = S[t]
                g, ti = divmod(t, G)
                vt = st["vg"][:, :, ti * T : (ti + 1) * T]
                pss = psH.tile([128, T], F, tag="h", name=f"pss{t}")
                nc.tensor.matmul(pss[:], w["wss_0"][:], st["g2a"][:],
                                 start=True, stop=True)
                ss = ss_p.tile([128, T], F16, tag="ss", name=f"ss{t}")
                nc.vector.tensor_scalar_add(ss[:], pss[:], bias["bss_0"][:])
                vnew = vn_p.tile([128, 3, T], F16, tag="vn", name=f"vn{t}")
                sq = sqb_p.tile([128, 3, T], F16, tag="sqb", name=f"sqb{t}")
                for c in range(3):
                    pv1 = psV.tile([128, T], F, tag="v", name=f"pv1{t}_{c}")
                    nc.tensor.matmul(pv1[:], w["wv1_0"][:], vt[:, c, :],
                                     start=True, stop=True)
                    nc.vector.tensor_mul(vnew[:, c, :], pv1[:], ss[:])
                for c in range(3):
                    pv2 = psV.tile([128, T], F, tag="v", name=f"pv2b{t}_{c}")
                    nc.tensor.matmul(pv2[:], w["wv2_1"][:], vnew[:, c, :],
                                     start=True, stop=True)
                    nc.scalar.activation(sq[:, c, :], pv2[:], AF.Square)
                n01 = n01_p.tile([128, T], F16, tag="n01", name=f"n01b{t}")
                nc.gpsimd.tensor_add(n01[:], sq[:, 0, :], sq[:, 1, :])
                n2 = n2b_p.tile([128, T], F16, tag="n2b", name=f"n2b{t}")
                nc.gpsimd.tensor_add(n2[:], n01[:], sq[:, 2, :])
                st["n2b"] = n2

            def stage_finish(t):
                """After silu(g2b): z and masked-pool accumulate."""
                st = S[t]
                pz = psH.tile([128, 4], F, tag="h", name=f"pz{t}")
                for j in range(4):
                    nc.tensor.matmul(
                        pz[:, j : j + 1],
                        st["g2b"][:, j * 128 : (j + 1) * 128],
                        weff[:],
                        start=True, stop=True, skip_group_check=True,
                    )
                zt = zz_p.tile([128, 4], F16, tag="zt", name=f"zt{t}")
                nc.vector.tensor_scalar_add(zt[:], pz[:], bias["beff"][:])
                for j in range(4):
                    nc.tensor.matmul(
                        ypart[:],
                        zt[:, j : j + 1],
                        st["bt"][:, j, :],
                        start=(pool_mm[0] == 0),
                        stop=(pool_mm[0] == 4 * NT - 1),
                        skip_group_check=True,
                    )
                    pool_mm[0] += 1
                del S[t]["g2b"]
                del S[t]["bt"]
                del S[t]

            def dma_group(q):
                a0 = q * GT
                vg = vin_p.tile([128, 3, GT], F16, tag="vg", name=f"vg{q}")
                nc.sync.dma_start(out=vg[:], in_=vT[:, :, a0 : a0 + GT])
                sg_in = sin_p.tile([128, GT], F16, tag="sg_in", name=f"sg{q}")
                nc.sync.dma_start(out=sg_in[:], in_=sT[:, a0 : a0 + GT])
                for t in range(q * G, (q + 1) * G):
                    S[t] = {"vg": vg, "sg_in": sg_in}

            # ---- main loop: one iteration per group, software-pipelined.
            # The next group's DMA + block-0 head is emitted BEFORE this
            # group's tails so its squares/adds fill the boundary bubble.
            dma_group(0)
            for t in range(G):
                stage_pre(t)
            for g in range(NG + 1):
                tb = list(range(g * G, (g + 1) * G)) if g < NG else []
                tcq = list(range((g - 1) * G, g * G)) if g > 0 else []

                for t in tcq:
                    bt = bin_p.tile([128, 4, B], F16, tag="bt", name=f"bt{t}")
                    nc.sync.dma_start(
                        out=bt[:],
                        in_=bT[t * T : (t + 1) * T, :].rearrange(
                            "(j p) b -> p j b", p=128
                        ),
                    )
                    S[t]["bt"] = bt

                # ---- sqrt phase (one table load) ----
                norms0, norms1 = {}, {}
                for t in tb:
                    nm = nrm_p.tile([128, T], F16, tag="nrm0", name=f"nrm0_{t}")
                    _pin(nc.scalar.activation(nm[:], S[t]["n2a"][:], AF.Sqrt))
                    norms0[t] = nm
                for t in tcq:
                    nm = nrm_p.tile([128, T], F16, tag="nrm1", name=f"nrm1_{t}")
                    _pin(nc.scalar.activation(nm[:], S[t]["n2b"][:], AF.Sqrt))
                    norms1[t] = nm
                _end_phase()

                # ---- h1 matmuls + silu phase (one table load) ----
                # B/C interleaved per index so psH slots recycle tilewise
                for k in range(G):
                    if k < len(tb):
                        t = tb[k]
                        g_, ti = divmod(t, G)
                        stt = S[t]["sg_in"][:, ti * T : (ti + 1) * T]
                        ph = psH.tile([128, T], F, tag="h", name=f"ph1a{t}")
                        nc.tensor.matmul(ph[:], w["wh1s_0"][:], stt,
                                         start=True, stop=False,
                                         skip_group_check=True)
                        nc.tensor.matmul(ph[:], w["wh1n_0"][:], norms0[t][:],
                                         start=False, stop=True,
                                         skip_group_check=True)
                        g2a = g2a_p.tile([128, T], F16, tag="g2a", name=f"g2a{t}")
                        _pin(nc.scalar.activation(g2a[:], ph[:], AF.Silu,
                                                  bias=bias["b1_0"][:]))
                        S[t]["g2a"] = g2a
                    if k < len(tcq):
                        t = tcq[k]
                        ph = psH.tile([128, T], F, tag="h", name=f"ph1b{t}")
                        nc.tensor.matmul(ph[:], w["wh1sg_1"][:], S[t]["g2a"][:],
                                         start=True, stop=False,
                                         skip_group_check=True)
                        nc.tensor.matmul(ph[:], w["wh1n_1"][:], norms1[t][:],
                                         start=False, stop=True,
                                         skip_group_check=True)
                        g2b = g2b_p.tile([128, T], F16, tag="g2b", name=f"g2b{t}")
                        _pin(nc.scalar.activation(g2b[:], ph[:], AF.Silu,
                                                  bias=bias["b1_1"][:]))
                        S[t]["g2b"] = g2b
                _end_phase()

                # ---- next group's head, then this group's tails ----
                if g + 1 < NG:
                    dma_group(g + 1)
                    for t in range((g + 1) * G, (g + 2) * G):
                        stage_pre(t)
                for t in tcq:
                    stage_finish(t)
                for t in tb:
                    stage_block0_tail(t)

            yout = yo_p.tile([1, B], F, tag="yout", name="yout")
            nc.vector.tensor_copy(yout[:], ypart[:])
            nc.sync.dma_start(out=y[:], in_=yout[:])

    _legalize_waits(nc)
    return nc


def _prep_weights(inputs):
    f32 = lambda a: np.asarray(a, np.float32)
    u0_w, v0_w = f32(inputs["u0_w"]), f32(inputs["v0_w"])
    a0_w1, a0_b1 = f32(inputs["a0_w1"]), f32(inputs["a0_b1"])
    a0_w2, a0_b2 = f32(inputs["a0_w2"]), f32(inputs["a0_b2"])
    v1_w = f32(inputs["v1_w"])
    a1_w1, a1_b1 = f32(inputs["a1_w1"]), f32(inputs["a1_b1"])
    a1_w2, a1_b2 = f32(inputs["a1_w2"]), f32(inputs["a1_b2"])
    out_w, out_b = f32(inputs["out_w"]), f32(inputs["out_b"])

    f16c = lambda a: np.ascontiguousarray(a, np.float16)
    f32c = lambda a: np.ascontiguousarray(a.reshape(NF, 1), np.float32)

    # composed block-1 s-path: h1b_sg = (W1s @ Wsg) @ g2a ; bias folded
    W1s = a1_w1[:, :NF]                  # (g_out, f_in) acting on block-1 s
    Wsg = a0_w2[:NF, :]                  # (f_out, j_in) block-0 sg head
    bsg = a0_b2[:NF]
    Wc = W1s @ Wsg                       # (g_out, j_in)
    b1_1_eff = a1_b1 + W1s @ bsg

    return {
        "wv1_0": f16c(u0_w.T),
        "wv2_0": f16c(v0_w.T),
        "wh1s_0": f16c(a0_w1.T[:NF]),
        "wh1n_0": f16c(a0_w1.T[NF:]),
        "wss_0": f16c(a0_w2[NF:].T),
        "wv2_1": f16c(v1_w.T),
        "wh1sg_1": f16c(Wc.T),
        "wh1n_1": f16c(a1_w1.T[NF:]),
        "weff": f16c((out_w[0] @ a1_w2[:NF]).reshape(NF, 1)),
        "b1_0": f32c(np.broadcast_to(a0_b1, (NF,)).copy()),
        "bss_0": f32c(a0_b2[NF:].copy()),
        "b1_1": f32c(b1_1_eff),
        "beff": np.full((NF, 1), float(out_w[0] @ a1_b2[:NF] + out_b[0]), np.float32),
        "eps": np.full((NF, 1), EPS, np.float32),
    }


def kernel(**inputs):
    _install_compat_patches()
    if os.environ.get("BASS_TRACE"):
        _maybe_install_trace_shim()
    from concourse.bass_utils import run_bass_kernel_spmd

    s = np.asarray(inputs["s"], np.float32)
    v = np.asarray(inputs["v"], np.float32)
    batch = np.asarray(inputs["batch"], np.float32)
    weights = _prep_weights(inputs)

    v0 = v[0]            # (NA, 3, NF)
    s0 = s[0]            # (NA, NF)
    bm = batch[:, :, 0]  # (B, NA)

    in_maps = []
    for c in range(NCORES):
        sl = slice(c * SH, (c + 1) * SH)
        vt = np.zeros((NF, 3, NAs), np.float16)
        vt[:, :, :SH] = v0[sl].transpose(2, 1, 0)
        st = np.zeros((NF, NAs), np.float16)
        st[:, :SH] = s0[sl].T
        bt = np.zeros((NAs, B), np.float16)
        bt[:SH] = bm[:, sl].T
        in_maps.append({"vT": vt, "sT": st, "bT": bt, **weights})

    key = "prog"
    if key not in _prog_cache:
        _prog_cache[key] = _build_program()
    nc = _prog_cache[key]

    res = run_bass_kernel_spmd(nc, in_maps, list(range(NCORES)))
    if res.exec_time_ns is not None:
        print(f"HW exec time: {res.exec_time_ns} ns")
    kernel._last_result = res

    ysum = np.zeros((B,), np.float64)
    for c in range(NCORES):
        ysum += res.results[c]["y"].reshape(B).astype(np.float64)
    return ysum.astype(np.float32).reshape(B, 1)
